# revision 3
# baseline (speedup 1.0000x reference)
"""Causal single-head attention (B=4, T=4096, D=1024, D_H=64) on 8 TRN2 cores.

Strategy (two SPMD Bass kernels with a host exchange between them):

Kernel 1 (projections), data-parallel over (batch, sequence-half):
  core 2b+h handles rows [h*2048, (h+1)*2048) of batch b and computes
  Q^T/K^T/V^T [64, 2048] in bf16 (fp32 PSUM accumulation). The host
  pre-transposes x into [d, t] layout so the kernel needs no on-device
  transposes (matmul contracts over the partition dim).

Kernel 2 (attention), causal-balanced and structurally uniform:
  per batch the causal T x T triangle is cut into
    tri-top  = rows 0..2047   x keys 0..2047   (causal triangle)
    tri-bot  = rows 2048..4095 x keys 2048..4095 (causal triangle)
    rect     = rows 2048..4095 x keys 0..2047  (dense)
  core 2b   gets tri-top + left  half of rect (keys 0..1023)
  core 2b+1 gets tri-bot + right half of rect (keys 1024..2047)
  Every core therefore runs the *identical* program: one 2048-causal
  triangle + one 2048x1024 dense rectangle = 72 (128-key x 512-query)
  chunk iterations. Per chunk: S^T = K_chunk @ Q^T on PE (scores with keys
  on partitions), exp on ACT (scale=1/32 folded into the activation),
  causal mask multiply on DVE for diagonal chunks only, then O^T += V'^T @ P^T
  on PE where V' carries an appended ones-column so the softmax denominator
  drops out of the same matmul. The host performs the final divide and the
  3-way combine for the bottom rows. Scores are bounded (|s| <~ 0.5) so
  plain exp without max-subtraction is numerically safe.
"""

import numpy as np
import ml_dtypes

import concourse.bass as bass
import concourse.tile as tile
import concourse.mybir as mybir
from concourse.bass import ts
from concourse.bass_utils import run_bass_kernel_spmd

BF16_NP = ml_dtypes.bfloat16
BF16 = mybir.dt.bfloat16
FP32 = mybir.dt.float32

B, T, D, DH = 4, 4096, 1024, 64
HALF = T // 2            # 2048 rows per projection shard / per piece
NCORES = 8
SCALE = float(D) ** -0.5  # 1/32, applied inside the exp activation

# ---------------------------------------------------------------------------
# Workaround: this walrus build rejects instructions carrying more than one
# sync wait ("Too many sync wait commands" in setupSyncWait). Tile's
# add_semaphores stage attaches up to ~3 waits per instruction. Post-pass:
# hoist all but the last wait of every instruction into preceding same-engine
# single-wait NoOps (engines execute their stream in order, so this is
# semantically identical).
# ---------------------------------------------------------------------------
def _split_sync_waits(nc):
    for fn in nc.m.functions:
        for bb in fn.blocks:
            insts = list(bb.instructions)
            out, ctr = [], 0
            for inst in insts:
                si = inst.sync_info
                waits = list(si.on_wait) if (si is not None and si.on_wait) else []
                if len(waits) > 1:
                    for w in waits[:-1]:
                        nop = mybir.InstNoOp(
                            name=f"{inst.name}__swait{ctr}",
                            engine=inst.engine,
                            ins=[],
                            outs=[],
                            sync_info=mybir.SyncInfo(on_wait=[w], on_update=[]),
                        )
                        out.append(nop)
                        ctr += 1
                    inst.sync_info = mybir.SyncInfo(
                        on_wait=[waits[-1]],
                        on_update=list(si.on_update or []),
                    )
                out.append(inst)
            if ctr:
                bb.instructions = out


# ---------------------------------------------------------------------------
# Kernel 1: fused QKV projection for a 2048-row shard of x.
# ---------------------------------------------------------------------------
def build_proj():
    nc = bass.Bass()
    # xT[tb, p, dc, t] = x_shard[tb*512 + t, dc*128 + p]
    xT = nc.dram_tensor("xT", [4, 128, 8, 512], BF16, kind="ExternalInput")
    # W[p, dc, j, h] = W_j[dc*128 + p, h], j in {Q, K, V}
    W = nc.dram_tensor("W", [128, 8, 3, 64], BF16, kind="ExternalInput")
    qkvT = nc.dram_tensor("qkvT", [3, 64, HALF], BF16, kind="ExternalOutput")

    with tile.TileContext(nc) as tc:
        with (
            tc.tile_pool(name="wpool", bufs=1) as wpool,
            tc.tile_pool(name="xpool", bufs=2) as xpool,
            tc.tile_pool(name="opool", bufs=3) as opool,
            tc.tile_pool(name="psum", bufs=3, space="PSUM") as psum,
        ):
            w_sb = wpool.tile([128, 8, 3, 64], BF16)
            nc.sync.dma_start(out=w_sb, in_=W[:])
            for tb in range(4):
                x_sb = xpool.tile([128, 8, 512], BF16)
                nc.sync.dma_start(out=x_sb, in_=xT[tb])
                for j in range(3):
                    ps = psum.tile([64, 512], FP32)
                    for dc in range(8):
                        nc.tensor.matmul(
                            ps,
                            lhsT=w_sb[:, dc, j, :],
                            rhs=x_sb[:, dc, :],
                            start=(dc == 0),
                            stop=(dc == 7),
                        )
                    o_sb = opool.tile([64, 512], BF16)
                    nc.vector.tensor_copy(out=o_sb, in_=ps)
                    nc.sync.dma_start(out=qkvT[j, :, ts(tb, 512)], in_=o_sb)
    _split_sync_waits(nc)
    return nc


# ---------------------------------------------------------------------------
# Kernel 2: attention for one 2048-causal-triangle + one 2048x1024 rectangle.
# ---------------------------------------------------------------------------
TRI_NKC = [4, 8, 12, 16]   # 128-key chunks per 512-query block (causal)
RECT_NKC = [8, 8, 8, 8]    # dense rectangle: 1024 keys
GROUP = 3                  # S chunks exp'd per ACT instruction (3 PSUM banks)


def build_attn():
    nc = bass.Bass()
    qt_tri = nc.dram_tensor("qt_tri", [128, HALF], BF16, kind="ExternalInput")
    kt_tri = nc.dram_tensor("kt_tri", [128, HALF], BF16, kind="ExternalInput")
    vp_tri = nc.dram_tensor("vp_tri", [128, 16, 65], BF16, kind="ExternalInput")
    qt_rect = nc.dram_tensor("qt_rect", [128, HALF], BF16, kind="ExternalInput")
    kt_rect = nc.dram_tensor("kt_rect", [128, 1024], BF16, kind="ExternalInput")
    vp_rect = nc.dram_tensor("vp_rect", [128, 8, 65], BF16, kind="ExternalInput")
    # masks[k, j, q] = 1.0 if q >= j*128 + k else 0.0
    masks = nc.dram_tensor("masks", [128, 4, 512], BF16, kind="ExternalInput")
    out_tri = nc.dram_tensor("out_tri", [4, 65, 512], FP32, kind="ExternalOutput")
    out_rect = nc.dram_tensor("out_rect", [4, 65, 512], FP32, kind="ExternalOutput")

    with tile.TileContext(nc) as tc:
        with (
            tc.tile_pool(name="const", bufs=1) as const,
            tc.tile_pool(name="ppool", bufs=3) as ppool,
            tc.tile_pool(name="osb", bufs=2) as osb,
            tc.tile_pool(name="spsum", bufs=2, space="PSUM") as spsum,
            tc.tile_pool(name="opsum", bufs=2, space="PSUM") as opsum,
        ):
            m_sb = const.tile([128, 4, 512], BF16, tag="masks")
            nc.sync.dma_start(out=m_sb, in_=masks[:])

            def load(ap, shape, tag):
                t = const.tile(shape, BF16, tag=tag)
                nc.sync.dma_start(out=t, in_=ap[:])
                return t

            qt_tri_sb = load(qt_tri, [128, HALF], "qt_tri")
            kt_tri_sb = load(kt_tri, [128, HALF], "kt_tri")
            vp_tri_sb = load(vp_tri, [128, 16, 65], "vp_tri")
            qt_rect_sb = load(qt_rect, [128, HALF], "qt_rect")
            kt_rect_sb = load(kt_rect, [128, 1024], "kt_rect")
            vp_rect_sb = load(vp_rect, [128, 8, 65], "vp_rect")

            def piece(qt_sb, kt_sb, vp_sb, nkc_list, diag, out_dram):
                for qb in range(4):
                    nkc = nkc_list[qb]
                    o_ps = opsum.tile([65, 512], FP32, tag="o_ps")
                    kc0 = 0
                    while kc0 < nkc:
                        gsz = min(GROUP, nkc - kc0)
                        s_ps = spsum.tile([128, GROUP, 512], FP32, tag="s_ps")
                        for i in range(gsz):
                            nc.tensor.matmul(
                                s_ps[:, i, :],
                                lhsT=kt_sb[:, ts(kc0 + i, 128)],
                                rhs=qt_sb[:, ts(qb, 512)],
                                start=True,
                                stop=True,
                            )
                        p_sb = ppool.tile([128, GROUP, 512], BF16, tag="p_sb")
                        nc.scalar.activation(
                            out=p_sb[:, :gsz, :],
                            in_=s_ps[:, :gsz, :],
                            func=mybir.ActivationFunctionType.Exp,
                            scale=SCALE,
                        )
                        if diag:
                            for i in range(gsz):
                                j = kc0 + i - (nkc - 4)
                                if 0 <= j < 4:
                                    nc.vector.tensor_mul(
                                        out=p_sb[:, i, :],
                                        in0=p_sb[:, i, :],
                                        in1=m_sb[:, j, :],
                                    )
                        for i in range(gsz):
                            kc = kc0 + i
                            nc.tensor.matmul(
                                o_ps,
                                lhsT=vp_sb[:, kc, :],
                                rhs=p_sb[:, i, :],
                                start=(kc == 0),
                                stop=(kc == nkc - 1),
                            )
                        kc0 += gsz
                    o_sb = osb.tile([65, 512], FP32, tag="o_sb")
                    nc.vector.tensor_copy(out=o_sb, in_=o_ps)
                    nc.sync.dma_start(out=out_dram[qb], in_=o_sb)

            piece(qt_tri_sb, kt_tri_sb, vp_tri_sb, TRI_NKC, True, out_tri)
            piece(qt_rect_sb, kt_rect_sb, vp_rect_sb, RECT_NKC, False, out_rect)
    _split_sync_waits(nc)
    return nc


_NCS = {}


def get_ncs():
    if not _NCS:
        _NCS["proj"] = build_proj()
        _NCS["attn"] = build_attn()
    return _NCS


def _build_masks():
    ki = np.arange(128)[:, None]
    qi = np.arange(512)[None, :]
    m = np.zeros((128, 4, 512), dtype=BF16_NP)
    for j in range(4):
        m[:, j, :] = (qi >= j * 128 + ki).astype(BF16_NP)
    return m


def _vp_chunks(v):
    """[n, 64] fp32/bf16 value rows -> [128, n/128, 65] bf16 with ones col."""
    n = v.shape[0]
    vp = np.ones((n, 65), dtype=BF16_NP)
    vp[:, :64] = v.astype(BF16_NP)
    return np.ascontiguousarray(vp.reshape(n // 128, 128, 65).transpose(1, 0, 2))


def _pad128(a):
    """[64, t] -> [128, t] zero-padded (contraction dim padded to 128)."""
    out = np.zeros((128, a.shape[1]), dtype=BF16_NP)
    out[:64] = a
    return out


def kernel(x, Wq, Wk, Wv):
    x = np.asarray(x, dtype=np.float32)
    ncs = get_ncs()
    core_ids = list(range(NCORES))

    # ---- kernel 1: projections -------------------------------------------
    W3 = np.stack(
        [np.asarray(Wq, np.float32), np.asarray(Wk, np.float32),
         np.asarray(Wv, np.float32)], axis=1,
    )  # [D, 3, 64]
    Wb = np.ascontiguousarray(
        W3.reshape(8, 128, 3, 64).transpose(1, 0, 2, 3)
    ).astype(BF16_NP)  # [p, dc, j, h]

    in1 = []
    for c in range(NCORES):
        b, hf = divmod(c, 2)
        xs = x[b, hf * HALF : (hf + 1) * HALF, :]  # [2048, 1024]
        xt = np.ascontiguousarray(
            xs.reshape(4, 512, 8, 128).transpose(0, 3, 2, 1)
        ).astype(BF16_NP)  # [tb, p, dc, t]
        in1.append({"xT": xt, "W": Wb})
    r1 = run_bass_kernel_spmd(ncs["proj"], in1, core_ids=core_ids).results

    # ---- host exchange ----------------------------------------------------
    QT, KT, V = {}, {}, {}
    for b in range(B):
        lo, hi = r1[2 * b]["qkvT"], r1[2 * b + 1]["qkvT"]
        QT[b] = np.concatenate([lo[0], hi[0]], axis=1)  # [64, 4096] bf16
        KT[b] = np.concatenate([lo[1], hi[1]], axis=1)
        V[b] = np.concatenate([lo[2], hi[2]], axis=1).T  # [4096, 64]

    masks = _build_masks()
    in2 = []
    for c in range(NCORES):
        b, hf = divmod(c, 2)
        if hf == 0:
            qt_tri, kt_tri = QT[b][:, :HALF], KT[b][:, :HALF]
            vp_tri = _vp_chunks(V[b][:HALF])
            qt_rect, kt_rect = QT[b][:, HALF:], KT[b][:, :1024]
            vp_rect = _vp_chunks(V[b][:1024])
        else:
            qt_tri, kt_tri = QT[b][:, HALF:], KT[b][:, HALF:]
            vp_tri = _vp_chunks(V[b][HALF:])
            qt_rect, kt_rect = QT[b][:, HALF:], KT[b][:, 1024:HALF]
            vp_rect = _vp_chunks(V[b][1024:HALF])
        in2.append(
            {
                "qt_tri": _pad128(qt_tri),
                "kt_tri": _pad128(kt_tri),
                "vp_tri": vp_tri,
                "qt_rect": _pad128(qt_rect),
                "kt_rect": _pad128(kt_rect),
                "vp_rect": vp_rect,
                "masks": masks,
            }
        )
    r2 = run_bass_kernel_spmd(ncs["attn"], in2, core_ids=core_ids).results

    # ---- host combine -----------------------------------------------------
    def flat(a):  # [4, 65, 512] -> [65, 2048]
        return np.ascontiguousarray(a.transpose(1, 0, 2)).reshape(65, HALF)

    out = np.empty((B, T, DH), dtype=np.float32)
    for b in range(B):
        t0 = flat(r2[2 * b]["out_tri"])
        out[b, :HALF] = (t0[:64] / t0[64]).T
        t1 = flat(r2[2 * b + 1]["out_tri"])
        r0 = flat(r2[2 * b]["out_rect"])
        rr = flat(r2[2 * b + 1]["out_rect"])
        num = t1[:64] + r0[:64] + rr[:64]
        den = t1[64] + r0[64] + rr[64]
        out[b, HALF:] = (num / den).T
    return out
